# revision 6
# baseline (speedup 1.0000x reference)
"""Trainium2 Bass kernel for BaseAttention (Bahdanau-style additive attention).

Reference computation (per batch row b):
    att_h  = h @ W.T + b_h                         # [B, A]
    dot    = tanh(iaf + att_h[:, None, :])         # [B, L, A]
    scores = dot @ alpha + alpha_b                 # [B, L]
    w      = softmax(scores, axis=1)               # [B, L]
    out    = sum_l w[b, l] * af[b, l, :]           # [B, D]

Sharding: data-parallel over batch, B=128 -> 16 per core across 8 cores.

The kernel is HBM-bandwidth bound (af alone is 25 MB/core in fp32), so all
large streams are cast to bf16 on the host: af+iaf are interleaved into one
pre-tiled stream `comb` ([P, NT, A+D], zero-padded to a whole number of
128-row tiles) so every DMA lands as large contiguous per-partition
descriptors.

Per-core device layout (rows = (b, l) flattened then zero-padded, RP=3200):
  - per 4-tile batch: att_h broadcast to tile rows via indicator matmuls
    (ind_t.T @ att_hb); all DVE/ACT elementwise work batched 4 tiles per
    instruction (DVE per-op fixed cost dominates at [128, 512] granularity).
  - softmax denominator deferred: e = exp(scores) unnormalized; the final
    result is (sum_l e*af) * 1/(sum_l e).
  - weighted sum over l is a matmul per (tile, d-chunk) using masked lhsT
    columns: e_cols[:, b] = e * indicator(row belongs to b).  The indicator
    is zero on pad rows, which also masks them out of the denominator.
"""

import os
from contextlib import ExitStack

import numpy as np
import ml_dtypes

import concourse.bass as bass
import concourse.mybir as mybir
import concourse.tile as tile
from concourse import bacc
from concourse.bass_utils import run_bass_kernel_spmd

F32 = mybir.dt.float32
BF16 = mybir.dt.bfloat16
AF_T = mybir.ActivationFunctionType
NPBF16 = ml_dtypes.bfloat16

B, L, D, A = 128, 196, 2048, 512
NCORES = 8
BPC = B // NCORES          # 16 batch rows per core
R = BPC * L                # 3136 valid (b, l) rows per core
P = 128                    # partitions
NT = (R + P - 1) // P      # 25 row tiles (tail zero-padded)
RP = NT * P                # 3200 padded rows
GT = 4                     # tiles per batch/DMA group: 4,4,4,4,4,4,1
KCH = D // P               # 16 k-chunks for the h @ W.T matmul
DCH = 4                    # d chunks of 512 for the weighted sum
DC = D // DCH              # 512
CW = A + D                 # 2560 combined (iaf | af) row width

GROUPS = []
_t = 0
while _t < NT:
    n = min(GT, NT - _t)
    GROUPS.append((_t, n))
    _t += n


def _build_program():
    nc = bacc.Bacc(None, target_bir_lowering=False)

    h_t = nc.declare_dram_parameter("h_t", [P, KCH * BPC], BF16, isOutput=False)
    w_t = nc.declare_dram_parameter("w_t", [P, KCH * A], BF16, isOutput=False)
    b_bc = nc.declare_dram_parameter("b_bc", [BPC, A], BF16, isOutput=False)
    alpha_bc = nc.declare_dram_parameter("alpha_bc", [P, A], BF16, isOutput=False)
    alphab_bc = nc.declare_dram_parameter("alphab_bc", [P, 1], F32, isOutput=False)
    ind = nc.declare_dram_parameter("ind", [P, NT * BPC], BF16, isOutput=False)
    ind_t = nc.declare_dram_parameter("ind_t", [BPC, RP], BF16, isOutput=False)
    comb = nc.declare_dram_parameter("comb", [P, NT * CW], BF16, isOutput=False)
    out = nc.declare_dram_parameter("out", [BPC, D], F32, isOutput=True)

    with ExitStack() as ctx:
        tc = ctx.enter_context(tile.TileContext(nc))
        consts = ctx.enter_context(tc.tile_pool(name="consts", bufs=1))
        stream = ctx.enter_context(tc.tile_pool(name="stream", bufs=3))
        scr = ctx.enter_context(tc.tile_pool(name="scr", bufs=2))
        ps_bc = ctx.enter_context(
            tc.tile_pool(name="ps_bc", bufs=1, space=bass.MemorySpace.PSUM)
        )
        ps_hb = ctx.enter_context(
            tc.tile_pool(name="ps_hb", bufs=1, space=bass.MemorySpace.PSUM)
        )
        ps_acc = ctx.enter_context(
            tc.tile_pool(name="ps_acc", bufs=1, space=bass.MemorySpace.PSUM)
        )

        # --- constants / weights ---
        w_sb = consts.tile([P, KCH, A], BF16)
        nc.sync.dma_start(w_sb[:], w_t[:, :].rearrange("p (k a) -> p k a", k=KCH))
        ht_sb = consts.tile([P, KCH, BPC], BF16)
        nc.sync.dma_start(ht_sb[:], h_t[:, :].rearrange("p (k b) -> p k b", k=KCH))
        bbc_sb = consts.tile([BPC, A], BF16)
        nc.sync.dma_start(bbc_sb[:], b_bc[:, :])
        abc_sb = consts.tile([P, A], BF16)
        nc.sync.dma_start(abc_sb[:], alpha_bc[:, :])
        abb_sb = consts.tile([P, 1], F32)
        nc.sync.dma_start(abb_sb[:], alphab_bc[:, :])
        ind_sb = consts.tile([P, NT, BPC], BF16)
        nc.sync.dma_start(ind_sb[:], ind[:, :].rearrange("p (t b) -> p t b", t=NT))
        indt_sb = consts.tile([BPC, RP], BF16)
        nc.sync.dma_start(indt_sb[:], ind_t[:, :])

        scores_all = consts.tile([P, NT], F32)
        e_all = consts.tile([P, NT], F32)
        ones_sb = consts.tile([P, 1], F32)
        nc.vector.memset(ones_sb[:], 1.0)
        eacc = consts.tile([P, GT, BPC], F32)
        nc.vector.memset(eacc[:], 0.0)

        # --- att_hb = h @ W.T + b_h, shape [BPC, A] ---
        atthb_ps = ps_hb.tile([BPC, A], F32, tag="misc")
        for k in range(KCH):
            nc.tensor.matmul(
                atthb_ps[:],
                ht_sb[:, k, :],
                w_sb[:, k, :],
                start=(k == 0),
                stop=(k == KCH - 1),
            )
        atthb_sb = consts.tile([BPC, A], BF16)
        nc.vector.tensor_add(atthb_sb[:], atthb_ps[:], bbc_sb[:])

        # --- weighted-sum accumulator ---
        acc_ps = ps_acc.tile([BPC, DCH, DC], F32)

        for t0, n in GROUPS:
            cur = stream.tile([P, GT, CW], BF16, tag="comb")
            nc.sync.dma_start(
                cur[:, :n, :],
                comb[:, t0 * CW : (t0 + n) * CW].rearrange("p (t c) -> p t c", t=n),
            )

            # att_hb broadcast to rows (paired to bound PSUM usage), then
            # tadd = iaf + bc in one DVE op per pair
            tadd = scr.tile([P, GT, A], BF16, tag="tadd")
            for p0 in range(0, n, 2):
                pn = min(2, n - p0)
                bc_ps = ps_bc.tile([P, 2, A], F32, tag="bc")
                for jj in range(pn):
                    tt = t0 + p0 + jj
                    nc.tensor.matmul(
                        bc_ps[:, jj, :],
                        indt_sb[:, tt * P : (tt + 1) * P],
                        atthb_sb[:],
                        start=True,
                        stop=True,
                    )
                nc.vector.tensor_add(
                    tadd[:, p0 : p0 + pn, :],
                    cur[:, p0 : p0 + pn, 0:A],
                    bc_ps[:, :pn, :],
                )

            tanh = scr.tile([P, GT, A], BF16, tag="tanh")
            nc.scalar.activation(tanh[:, :n, :], tadd[:, :n, :], AF_T.Tanh)

            # scores = sum_a tanh * alpha  (alpha_b folded into Exp bias)
            ttr = scr.tile([P, GT, A], BF16, tag="ttr")
            nc.vector.tensor_mul(
                ttr[:, :n, :],
                tanh[:, :n, :],
                abc_sb[:, :].unsqueeze(1).broadcast_to([P, n, A]),
            )
            nc.vector.tensor_reduce(
                scores_all[:, t0 : t0 + n],
                ttr[:, :n, :],
                axis=mybir.AxisListType.X,
                op=mybir.AluOpType.add,
            )
            nc.scalar.activation(
                e_all[:, t0 : t0 + n],
                scores_all[:, t0 : t0 + n],
                AF_T.Exp,
                bias=abb_sb[:],
            )

            # masked weight columns: e_cols[:, j, b] = e * (row belongs to b)
            ecols = scr.tile([P, GT, BPC], BF16, tag="ecols")
            nc.vector.tensor_mul(
                ecols[:, :n, :],
                ind_sb[:, t0 : t0 + n, :],
                e_all[:, t0 : t0 + n].unsqueeze(2).broadcast_to([P, n, BPC]),
            )
            nc.vector.tensor_add(eacc[:, :n, :], eacc[:, :n, :], ecols[:, :n, :])

            for j in range(n):
                for c in range(DCH):
                    nc.tensor.matmul(
                        acc_ps[:, c, :],
                        ecols[:, j, :],
                        cur[:, j, A + c * DC : A + (c + 1) * DC],
                        start=(t0 + j == 0),
                        stop=(t0 + j == NT - 1),
                    )

        # --- softmax denominator, normalize, store ---
        sums_ps = ps_hb.tile([BPC, 1], F32, tag="misc")
        for j in range(GT):
            nc.tensor.matmul(
                sums_ps[:],
                eacc[:, j, :],
                ones_sb[:],
                start=(j == 0),
                stop=(j == GT - 1),
            )
        recip = consts.tile([BPC, 1], F32)
        nc.vector.reciprocal(recip[:], sums_ps[:])
        out_sb = consts.tile([BPC, DCH, DC], F32)
        for c in range(DCH):
            nc.vector.tensor_scalar_mul(out_sb[:, c, :], acc_ps[:, c, :], recip[:])
            nc.sync.dma_start(out[:, c * DC : (c + 1) * DC], out_sb[:, c, :])

    nc.compile()
    return nc


_PROGRAM = None


def _get_program():
    global _PROGRAM
    if _PROGRAM is None:
        _PROGRAM = _build_program()
    return _PROGRAM


def _host_prep(h, att_feats, internal_att_feats, h2att_w, h2att_b, alpha_w, alpha_b):
    h = np.asarray(h, np.float32)
    att_feats = np.asarray(att_feats, np.float32)
    iaf = np.asarray(internal_att_feats, np.float32)
    h2att_w = np.asarray(h2att_w, np.float32)
    h2att_b = np.asarray(h2att_b, np.float32)
    alpha_w = np.asarray(alpha_w, np.float32)
    alpha_b = np.asarray(alpha_b, np.float32)

    # W.T [D, A] pre-tiled to [P, KCH*A]: w_t[p, k*A+a] = W[a, k*128+p]
    w_t = np.ascontiguousarray(
        h2att_w.T.reshape(KCH, P, A).transpose(1, 0, 2).reshape(P, KCH * A)
    ).astype(NPBF16)
    b_bc = np.tile(h2att_b.reshape(1, A), (BPC, 1)).astype(NPBF16)
    alpha_bc = np.tile(alpha_w.reshape(1, A), (P, 1)).astype(NPBF16)
    alphab_bc = np.full((P, 1), float(alpha_b.reshape(-1)[0]), np.float32)

    # row -> batch indicator over the padded row space (0 on pad rows)
    rows = np.arange(RP)
    onehot = np.zeros((RP, BPC), np.float32)
    valid = rows < R
    onehot[rows[valid], rows[valid] // L] = 1.0
    ind_arr = (
        onehot.reshape(NT, P, BPC).transpose(1, 0, 2).reshape(P, NT * BPC)
    ).astype(NPBF16)
    ind_t = np.ascontiguousarray(onehot.T).astype(NPBF16)

    in_maps = []
    for i in range(NCORES):
        sl = slice(i * BPC, (i + 1) * BPC)
        h_t = np.ascontiguousarray(
            h[sl].T.reshape(KCH, P, BPC).transpose(1, 0, 2).reshape(P, KCH * BPC)
        ).astype(NPBF16)

        iaf_rows = np.zeros((RP, A), np.float32)
        iaf_rows[:R] = iaf[sl].reshape(R, A)
        af_rows = np.zeros((RP, D), np.float32)
        af_rows[:R] = att_feats[sl].reshape(R, D)
        comb = np.empty((P, NT, CW), NPBF16)
        comb[:, :, :A] = iaf_rows.reshape(NT, P, A).transpose(1, 0, 2).astype(NPBF16)
        comb[:, :, A:] = af_rows.reshape(NT, P, D).transpose(1, 0, 2).astype(NPBF16)

        in_maps.append(
            {
                "h_t": h_t,
                "w_t": w_t,
                "b_bc": b_bc,
                "alpha_bc": alpha_bc,
                "alphab_bc": alphab_bc,
                "ind": ind_arr,
                "ind_t": ind_t,
                "comb": comb.reshape(P, NT * CW),
            }
        )
    return in_maps


def run(trace=False, **inputs):
    """Run the SPMD kernel; returns (full_output [B, D], BassKernelResults)."""
    nc = _get_program()
    in_maps = _host_prep(**inputs)
    res = run_bass_kernel_spmd(nc, in_maps, list(range(NCORES)), trace=trace)
    out = np.concatenate([res.results[i]["out"] for i in range(NCORES)], axis=0)
    return out, res


def kernel(**inputs):
    out, _ = run(trace=False, **inputs)
    return out


# revision 9
# speedup vs baseline: 1.3931x; 1.3931x over previous
"""Trainium2 Bass kernel for BaseAttention (Bahdanau-style additive attention).

Reference computation (per batch row b):
    att_h  = h @ W.T + b_h                         # [B, A]
    dot    = tanh(iaf + att_h[:, None, :])         # [B, L, A]
    scores = dot @ alpha + alpha_b                 # [B, L]
    w      = softmax(scores, axis=1)               # [B, L]
    out    = sum_l w[b, l] * af[b, l, :]           # [B, D]

Sharding: data-parallel over batch, B=128 -> 16 per core across 8 cores.

The kernel is HBM-bandwidth bound (af alone is 25 MB/core in fp32), so all
large streams are cast to bf16 on the host: af+iaf are interleaved into one
pre-tiled stream `comb` ([P, NT, A+D], zero-padded to a whole number of
128-row tiles) so every DMA lands as large contiguous per-partition
descriptors.

Per-core device layout (rows = (b, l) flattened then zero-padded, RP=3200):
  - per 4-tile batch: att_h broadcast to tile rows via indicator matmuls
    (ind_t.T @ att_hb); all DVE/ACT elementwise work batched 4 tiles per
    instruction (DVE per-op fixed cost dominates at [128, 512] granularity).
  - softmax denominator deferred: e = exp(scores) unnormalized; the final
    result is (sum_l e*af) * 1/(sum_l e).
  - weighted sum over l is a matmul per (tile, d-chunk) using masked lhsT
    columns: e_cols[:, b] = e * indicator(row belongs to b).  The indicator
    is zero on pad rows, which also masks them out of the denominator.
"""

import os
from contextlib import ExitStack

import numpy as np
import ml_dtypes

import concourse.bass as bass
import concourse.mybir as mybir
import concourse.tile as tile
from concourse import bacc
from concourse.bass_utils import run_bass_kernel_spmd

F32 = mybir.dt.float32
BF16 = mybir.dt.bfloat16
AF_T = mybir.ActivationFunctionType
NPBF16 = ml_dtypes.bfloat16

B, L, D, A = 128, 196, 2048, 512
NCORES = 8
BPC = B // NCORES          # 16 batch rows per core
R = BPC * L                # 3136 valid (b, l) rows per core
P = 128                    # partitions
NT = (R + P - 1) // P      # 25 row tiles (tail zero-padded)
RP = NT * P                # 3200 padded rows
GT = 4                     # tiles per batch/DMA group: 4,4,4,4,4,4,1
KCH = D // P               # 16 k-chunks for the h @ W.T matmul
DCH = 4                    # d chunks of 512 for the weighted sum
DC = D // DCH              # 512
CW = A + D                 # 2560 combined (iaf | af) row width

GROUPS = []
_t = 0
while _t < NT:
    n = min(GT, NT - _t)
    GROUPS.append((_t, n))
    _t += n


def _build_program():
    nc = bacc.Bacc(None, target_bir_lowering=False)

    h_t = nc.declare_dram_parameter("h_t", [P, KCH * BPC], BF16, isOutput=False)
    w_t = nc.declare_dram_parameter("w_t", [P, KCH * A], BF16, isOutput=False)
    b_bc = nc.declare_dram_parameter("b_bc", [BPC, A], BF16, isOutput=False)
    alpha_bc = nc.declare_dram_parameter("alpha_bc", [P, A], BF16, isOutput=False)
    alphab_bc = nc.declare_dram_parameter("alphab_bc", [P, 1], F32, isOutput=False)
    ind = nc.declare_dram_parameter("ind", [P, NT * BPC], BF16, isOutput=False)
    ind_t = nc.declare_dram_parameter("ind_t", [BPC, RP], BF16, isOutput=False)
    comb = nc.declare_dram_parameter("comb", [P, NT * CW], BF16, isOutput=False)
    out = nc.declare_dram_parameter("out", [BPC, D], F32, isOutput=True)

    with ExitStack() as ctx:
        tc = ctx.enter_context(tile.TileContext(nc))
        consts = ctx.enter_context(tc.tile_pool(name="consts", bufs=1))
        stream = ctx.enter_context(tc.tile_pool(name="stream", bufs=3))
        scr = ctx.enter_context(tc.tile_pool(name="scr", bufs=2))
        ps_bc = ctx.enter_context(
            tc.tile_pool(name="ps_bc", bufs=3, space=bass.MemorySpace.PSUM)
        )
        ps_hb = ctx.enter_context(
            tc.tile_pool(name="ps_hb", bufs=1, space=bass.MemorySpace.PSUM)
        )
        ps_acc = ctx.enter_context(
            tc.tile_pool(name="ps_acc", bufs=1, space=bass.MemorySpace.PSUM)
        )

        # --- constants / weights ---
        w_sb = consts.tile([P, KCH, A], BF16)
        nc.sync.dma_start(w_sb[:], w_t[:, :].rearrange("p (k a) -> p k a", k=KCH))
        ht_sb = consts.tile([P, KCH, BPC], BF16)
        nc.sync.dma_start(ht_sb[:], h_t[:, :].rearrange("p (k b) -> p k b", k=KCH))
        bbc_sb = consts.tile([BPC, A], BF16)
        nc.sync.dma_start(bbc_sb[:], b_bc[:, :])
        abc_sb = consts.tile([P, A], BF16)
        nc.sync.dma_start(abc_sb[:], alpha_bc[:, :])
        abb_sb = consts.tile([P, 1], F32)
        nc.sync.dma_start(abb_sb[:], alphab_bc[:, :])
        ind_sb = consts.tile([P, NT, BPC], BF16)
        nc.sync.dma_start(ind_sb[:], ind[:, :].rearrange("p (t b) -> p t b", t=NT))
        indt_sb = consts.tile([BPC, RP], BF16)
        nc.sync.dma_start(indt_sb[:], ind_t[:, :])

        scores_all = consts.tile([P, NT], F32)
        e_all = consts.tile([P, NT], F32)
        ones_sb = consts.tile([P, 1], F32)
        nc.vector.memset(ones_sb[:], 1.0)
        eacc = consts.tile([P, GT, BPC], F32)
        nc.vector.memset(eacc[:], 0.0)

        # --- att_hb = h @ W.T + b_h, shape [BPC, A] ---
        atthb_ps = ps_hb.tile([BPC, A], F32, tag="misc")
        for k in range(KCH):
            nc.tensor.matmul(
                atthb_ps[:],
                ht_sb[:, k, :],
                w_sb[:, k, :],
                start=(k == 0),
                stop=(k == KCH - 1),
            )
        atthb_sb = consts.tile([BPC, A], BF16)
        nc.vector.tensor_add(atthb_sb[:], atthb_ps[:], bbc_sb[:])

        # --- weighted-sum accumulator ---
        acc_ps = ps_acc.tile([BPC, DCH, DC], F32)

        for t0, n in GROUPS:
            cur = stream.tile([P, GT, CW], BF16, tag="comb")
            nc.sync.dma_start(
                cur[:, :n, :],
                comb[:, t0 * CW : (t0 + n) * CW].rearrange("p (t c) -> p t c", t=n),
            )

            # per tile: att_hb broadcast (matmul), iaf add (DVE, contiguous
            # PSUM operand), tanh (ACT) -- all [128, 512] granularity
            tanh = scr.tile([P, GT, A], BF16, tag="tanh")
            for j in range(n):
                tt = t0 + j
                bc_ps = ps_bc.tile([P, A], F32, tag="bc")
                nc.tensor.matmul(
                    bc_ps[:],
                    indt_sb[:, tt * P : (tt + 1) * P],
                    atthb_sb[:],
                    start=True,
                    stop=True,
                )
                tadd = scr.tile([P, A], BF16, tag="tadd")
                nc.vector.tensor_add(tadd[:], cur[:, j, 0:A], bc_ps[:])
                nc.scalar.activation(tanh[:, j, :], tadd[:], AF_T.Tanh)

            # scores = sum_a tanh * alpha  (alpha_b folded into Exp bias)
            ttr = scr.tile([P, GT, A], BF16, tag="ttr")
            nc.vector.tensor_mul(
                ttr[:, :n, :],
                tanh[:, :n, :],
                abc_sb[:, :].unsqueeze(1).broadcast_to([P, n, A]),
            )
            nc.vector.tensor_reduce(
                scores_all[:, t0 : t0 + n],
                ttr[:, :n, :],
                axis=mybir.AxisListType.X,
                op=mybir.AluOpType.add,
            )
            nc.scalar.activation(
                e_all[:, t0 : t0 + n],
                scores_all[:, t0 : t0 + n],
                AF_T.Exp,
                bias=abb_sb[:],
            )

            # masked weight columns: e_cols[:, j, b] = e * (row belongs to b)
            ecols = scr.tile([P, GT, BPC], BF16, tag="ecols")
            nc.vector.tensor_mul(
                ecols[:, :n, :],
                ind_sb[:, t0 : t0 + n, :],
                e_all[:, t0 : t0 + n].unsqueeze(2).broadcast_to([P, n, BPC]),
            )
            nc.vector.tensor_add(eacc[:, :n, :], eacc[:, :n, :], ecols[:, :n, :])

            for j in range(n):
                for c in range(DCH):
                    nc.tensor.matmul(
                        acc_ps[:, c, :],
                        ecols[:, j, :],
                        cur[:, j, A + c * DC : A + (c + 1) * DC],
                        start=(t0 + j == 0),
                        stop=(t0 + j == NT - 1),
                    )

        # --- softmax denominator, normalize, store ---
        sums_ps = ps_hb.tile([BPC, 1], F32, tag="misc")
        for j in range(GT):
            nc.tensor.matmul(
                sums_ps[:],
                eacc[:, j, :],
                ones_sb[:],
                start=(j == 0),
                stop=(j == GT - 1),
            )
        recip = consts.tile([BPC, 1], F32)
        nc.vector.reciprocal(recip[:], sums_ps[:])
        out_sb = consts.tile([BPC, DCH, DC], F32)
        nc.vector.tensor_scalar_mul(out_sb[:, :, :], acc_ps[:, :, :], recip[:])
        nc.sync.dma_start(
            out[:, :], out_sb[:, :, :].rearrange("b c d -> b (c d)")
        )

    nc.compile()
    return nc


_PROGRAM = None


def _get_program():
    global _PROGRAM
    if _PROGRAM is None:
        _PROGRAM = _build_program()
    return _PROGRAM


def _host_prep(h, att_feats, internal_att_feats, h2att_w, h2att_b, alpha_w, alpha_b):
    h = np.asarray(h, np.float32)
    att_feats = np.asarray(att_feats, np.float32)
    iaf = np.asarray(internal_att_feats, np.float32)
    h2att_w = np.asarray(h2att_w, np.float32)
    h2att_b = np.asarray(h2att_b, np.float32)
    alpha_w = np.asarray(alpha_w, np.float32)
    alpha_b = np.asarray(alpha_b, np.float32)

    # W.T [D, A] pre-tiled to [P, KCH*A]: w_t[p, k*A+a] = W[a, k*128+p]
    w_t = np.ascontiguousarray(
        h2att_w.T.reshape(KCH, P, A).transpose(1, 0, 2).reshape(P, KCH * A)
    ).astype(NPBF16)
    b_bc = np.tile(h2att_b.reshape(1, A), (BPC, 1)).astype(NPBF16)
    alpha_bc = np.tile(alpha_w.reshape(1, A), (P, 1)).astype(NPBF16)
    alphab_bc = np.full((P, 1), float(alpha_b.reshape(-1)[0]), np.float32)

    # row -> batch indicator over the padded row space (0 on pad rows)
    rows = np.arange(RP)
    onehot = np.zeros((RP, BPC), np.float32)
    valid = rows < R
    onehot[rows[valid], rows[valid] // L] = 1.0
    ind_arr = (
        onehot.reshape(NT, P, BPC).transpose(1, 0, 2).reshape(P, NT * BPC)
    ).astype(NPBF16)
    ind_t = np.ascontiguousarray(onehot.T).astype(NPBF16)

    in_maps = []
    for i in range(NCORES):
        sl = slice(i * BPC, (i + 1) * BPC)
        h_t = np.ascontiguousarray(
            h[sl].T.reshape(KCH, P, BPC).transpose(1, 0, 2).reshape(P, KCH * BPC)
        ).astype(NPBF16)

        iaf_rows = np.zeros((RP, A), np.float32)
        iaf_rows[:R] = iaf[sl].reshape(R, A)
        af_rows = np.zeros((RP, D), np.float32)
        af_rows[:R] = att_feats[sl].reshape(R, D)
        comb = np.empty((P, NT, CW), NPBF16)
        comb[:, :, :A] = iaf_rows.reshape(NT, P, A).transpose(1, 0, 2).astype(NPBF16)
        comb[:, :, A:] = af_rows.reshape(NT, P, D).transpose(1, 0, 2).astype(NPBF16)

        in_maps.append(
            {
                "h_t": h_t,
                "w_t": w_t,
                "b_bc": b_bc,
                "alpha_bc": alpha_bc,
                "alphab_bc": alphab_bc,
                "ind": ind_arr,
                "ind_t": ind_t,
                "comb": comb.reshape(P, NT * CW),
            }
        )
    return in_maps


def run(trace=False, **inputs):
    """Run the SPMD kernel; returns (full_output [B, D], BassKernelResults)."""
    nc = _get_program()
    in_maps = _host_prep(**inputs)
    res = run_bass_kernel_spmd(nc, in_maps, list(range(NCORES)), trace=trace)
    out = np.concatenate([res.results[i]["out"] for i in range(NCORES)], axis=0)
    return out, res


def kernel(**inputs):
    out, _ = run(trace=False, **inputs)
    return out
